# revision 5
# baseline (speedup 1.0000x reference)
"""CrossAttention TRN2 kernel.

Problem (hardcoded shapes):
  x    [4, 2048, 1024], cond [4, 2048, 1024]
  Wq/Wk/Wv [1024, 1024], Wo [1024, 1024], bo [1024]
  out = softmax((x@Wq) per 8 heads of 128 @ (cond@Wk)^T * 0.125) @ (cond@Wv) @ Wo + bo

Sharding: 8 cores = (batch b in 0..3) x (head-half hh in 0..1).
Each core computes heads hh*4..hh*4+3 over ALL 2048 query rows of one batch.
K/V/Q projections only cover the core's 4 heads (inner cols hh*512..hh*512+512),
so no projection work is replicated. The out-projection contracts only the
core's 512 inner dims, producing a PARTIAL [2048, 1024] output; the host sums
the two partials of each batch and adds bo. No collectives.

All matmul operands are bf16 (fp32 PSUM accumulation): full PE rate, half
LDWEIGHTS/SBUF/DMA cost vs fp32r. Scores stay transposed [j, i] so the
softmax denominator is a partition reduction (Pool engine) of the bf16
exp tiles accumulated on DVE (2x bf16 mode).

Per-head software pipeline: emit K-proj(h+1) after attention(h) so its
matmuls fill the PE stalls left by the exp (Act) dependency; out-proj of
the first i-half starts while the last head's second i-half attention runs.
"""
import numpy as np
import ml_dtypes

import concourse.bass as bass
import concourse.bacc as bacc
import concourse.tile as tile
from concourse import bass_isa, mybir
from concourse.bass_utils import run_bass_kernel_spmd

F32 = mybir.dt.float32
BF16 = mybir.dt.bfloat16
EXP = mybir.ActivationFunctionType.Exp

B, NQ, NK, D = 4, 2048, 2048, 1024   # D = query_dim = cond_dim = inner_dim = out_dim
H, DH = 8, 128                        # heads, per-head dim
HPC = 4                               # heads per core
GW = HPC * DH                         # 512 inner cols per core
SCALE = 64 ** -0.5                    # reference uses dim_head=64 for the scale
NCORES = 8
KT = D // 128                         # contraction tiles (8)
JT = NK // 128                        # key tiles (16)
IC = 1024                             # i-chunk per attention block
NIC = NQ // IC                        # 2


def build_nc():
    nc = bacc.Bacc()
    xT = nc.declare_dram_parameter("xT", [D, NQ], BF16, isOutput=False)
    condT = nc.declare_dram_parameter("condT", [D, NK], BF16, isOutput=False)
    wq = nc.declare_dram_parameter("wq", [D, GW], BF16, isOutput=False)
    wk = nc.declare_dram_parameter("wk", [D, GW], BF16, isOutput=False)
    wv = nc.declare_dram_parameter("wv", [D, GW], BF16, isOutput=False)
    wo = nc.declare_dram_parameter("wo", [GW, D], BF16, isOutput=False)
    out = nc.declare_dram_parameter("out", [NQ, D], F32, isOutput=True)

    with tile.TileContext(nc) as tc:
        with (
            nc.allow_low_precision(reason="bf16 matmul/softmax path is intended"),
            tc.tile_pool(name="wts", bufs=1) as wts,
            tc.tile_pool(name="big", bufs=1) as big,
            tc.tile_pool(name="xstream", bufs=3) as xstream,
            tc.tile_pool(name="expp", bufs=6) as expp,
            tc.tile_pool(name="denp", bufs=2) as denp,
            tc.tile_pool(name="ostage", bufs=2) as ostage,
            tc.tile_pool(name="ps", bufs=1, space="PSUM") as ps,
        ):
            # resident weights
            wq_s = wts.tile([128, KT, GW], BF16, tag="wq_s")
            wk_s = wts.tile([128, KT, GW], BF16, tag="wk_s")
            wv_s = wts.tile([128, KT, GW], BF16, tag="wv_s")
            wo_s = wts.tile([128, HPC, D], BF16, tag="wo_s")
            for k in range(KT):
                rows = slice(k * 128, (k + 1) * 128)
                nc.sync.dma_start(out=wk_s[:, k, :], in_=wk[rows, :])
                nc.sync.dma_start(out=wv_s[:, k, :], in_=wv[rows, :])
                nc.sync.dma_start(out=wq_s[:, k, :], in_=wq[rows, :])
            for h in range(HPC):
                nc.sync.dma_start(out=wo_s[:, h, :],
                                  in_=wo[h * 128:(h + 1) * 128, :])

            # resident condT: 8 tiles [128, 2048]
            ct = big.tile([128, KT, NK], BF16, tag="ct")
            for k in range(KT):
                nc.sync.dma_start(out=ct[:, k, :],
                                  in_=condT[k * 128:(k + 1) * 128, :])

            # per-head projections, attention output (all bf16, dh on partitions)
            kT_all = big.tile([128, HPC, NK], BF16, tag="kT_all")
            qT_all = big.tile([128, HPC, NQ], BF16, tag="qT_all")
            v_all = big.tile([128, JT, GW], BF16, tag="v_all")
            attT = big.tile([128, HPC, NQ], BF16, tag="attT")

            def k_proj(h):
                # kT_all[:, h, :] = (Wk_h)^T @ condT ; [dh=128, j=2048]
                for jp in range(NK // 1024):
                    acc = ps.tile([128, 1024], F32, tag="sc", bufs=2,
                                  name=f"kacc_{h}_{jp}")
                    for half in range(2):
                        jh = jp * 2 + half
                        for k in range(KT):
                            nc.tensor.matmul(
                                acc[:, half * 512:(half + 1) * 512],
                                wk_s[:, k, h * DH:(h + 1) * DH],
                                ct[:, k, jh * 512:(jh + 1) * 512],
                                start=(k == 0), stop=(k == KT - 1))
                    nc.vector.tensor_copy(
                        kT_all[:, h, jp * 1024:(jp + 1) * 1024], acc)

            def q_proj_all():
                # qT_all[:, h, :] = (Wq_h)^T @ xT for all 4 heads, x streamed once
                for ih in range(NQ // 512):
                    accs = [ps.tile([128, 1024], F32, tag="sc", bufs=2,
                                    name=f"qacc_{ih}_{hp}") for hp in range(2)]
                    xk_tiles = []
                    for k in range(KT):
                        xk = xstream.tile([128, 512], BF16, tag="xk",
                                          name=f"xk_{ih}_{k}")
                        nc.sync.dma_start(
                            out=xk,
                            in_=xT[k * 128:(k + 1) * 128, ih * 512:(ih + 1) * 512])
                        xk_tiles.append(xk)
                        for h in range(HPC):
                            nc.tensor.matmul(
                                accs[h // 2][:, (h % 2) * 512:(h % 2) * 512 + 512],
                                wq_s[:, k, h * DH:(h + 1) * DH],
                                xk,
                                start=(k == 0), stop=(k == KT - 1))
                    for h in range(HPC):
                        nc.vector.tensor_copy(
                            qT_all[:, h, ih * 512:(ih + 1) * 512],
                            accs[h // 2][:, (h % 2) * 512:(h % 2) * 512 + 512])

            def v_proj_all():
                # v_all[:, jt, :] = condT_jt^T @ Wv (all 4 heads); [j=128, 512]
                for jp in range(JT // 2):
                    acc = ps.tile([128, 1024], F32, tag="av", bufs=2,
                                  name=f"vacc_{jp}")
                    for half in range(2):
                        jt = jp * 2 + half
                        for k in range(KT):
                            nc.tensor.matmul(
                                acc[:, half * 512:(half + 1) * 512],
                                ct[:, k, jt * 128:(jt + 1) * 128],
                                wv_s[:, k, :],
                                start=(k == 0), stop=(k == KT - 1))
                    for half in range(2):
                        nc.vector.tensor_copy(
                            v_all[:, jp * 2 + half, :],
                            acc[:, half * 512:(half + 1) * 512])

            def attention(h):
                for ic in range(NIC):
                    i0 = ic * IC
                    avs = ps.tile([128, IC], F32, tag="av", bufs=2,
                                  name=f"avs_{h}_{ic}")
                    den_s = denp.tile([128, IC], BF16, tag="den_s",
                                      name=f"den_s_{h}_{ic}")
                    for jt in range(JT):
                        sc = ps.tile([128, IC], F32, tag="sc", bufs=2)
                        for ih in range(IC // 512):
                            nc.tensor.matmul(
                                sc[:, ih * 512:(ih + 1) * 512],
                                kT_all[:, h, jt * 128:(jt + 1) * 128],
                                qT_all[:, h, i0 + ih * 512:i0 + (ih + 1) * 512],
                                start=True, stop=True)
                        esc = expp.tile([128, IC], BF16, tag="esc")
                        nc.scalar.activation(esc, sc, EXP)
                        for ih in range(IC // 512):
                            nc.tensor.matmul(
                                avs[:, ih * 512:(ih + 1) * 512],
                                v_all[:, jt, h * DH:(h + 1) * DH],
                                esc[:, ih * 512:(ih + 1) * 512],
                                start=(jt == 0), stop=(jt == JT - 1))
                        if jt == 0:
                            nc.vector.tensor_copy(den_s, esc)
                        else:
                            nc.vector.tensor_add(den_s, den_s, esc)
                    den_bc = denp.tile([128, IC], F32, tag="den_bc",
                                       name=f"den_bc_{h}_{ic}")
                    nc.gpsimd.partition_all_reduce(
                        den_bc, den_s, 128, bass_isa.ReduceOp.add)
                    nc.vector.reciprocal_approx_fast(den_bc, den_bc)
                    nc.vector.tensor_mul(
                        attT[:, h, i0:i0 + IC], avs, den_bc)

            def out_proj(it):
                # out[it*128:(it+1)*128, :] = attT_it^T @ Wo  (partial; host adds bo)
                ot = ps.tile([128, D], F32, tag="av", bufs=2, name=f"ot_{it}")
                for nh in range(D // 512):
                    for h in range(HPC):
                        nc.tensor.matmul(
                            ot[:, nh * 512:(nh + 1) * 512],
                            attT[:, h, it * 128:(it + 1) * 128],
                            wo_s[:, h, nh * 512:(nh + 1) * 512],
                            start=(h == 0), stop=(h == HPC - 1))
                fo = ostage.tile([128, D], F32, tag="fo")
                nc.scalar.copy(fo, ot)
                nc.sync.dma_start(
                    out=out[it * 128:(it + 1) * 128, :], in_=fo)

            # emission order = scheduler priority: attention(h) before
            # k_proj(h+1) so projection matmuls fill attention's Act stalls.
            k_proj(0)
            v_proj_all()
            q_proj_all()
            attention(0)
            k_proj(1)
            attention(1)
            k_proj(2)
            attention(2)
            k_proj(3)
            attention(3)
            for it in range(NQ // 128):
                out_proj(it)
    nc.finalize()
    return nc


_NC_CACHE = None


def _get_nc():
    global _NC_CACHE
    if _NC_CACHE is None:
        _NC_CACHE = build_nc()
    return _NC_CACHE


def make_in_maps(x, cond, Wq, Wk, Wv, Wo):
    bf = ml_dtypes.bfloat16
    wq_s = (Wq.astype(np.float64) * SCALE).astype(bf)
    wk_c = Wk.astype(bf)
    wv_c = Wv.astype(bf)
    wo_c = Wo.astype(bf)
    in_maps = []
    for c in range(NCORES):
        b, hh = c // 2, c % 2
        cols = slice(hh * GW, (hh + 1) * GW)
        in_maps.append({
            "xT": np.ascontiguousarray(x[b].T.astype(bf)),
            "condT": np.ascontiguousarray(cond[b].T.astype(bf)),
            "wq": np.ascontiguousarray(wq_s[:, cols]),
            "wk": np.ascontiguousarray(wk_c[:, cols]),
            "wv": np.ascontiguousarray(wv_c[:, cols]),
            "wo": np.ascontiguousarray(wo_c[cols, :]),
        })
    return in_maps


def kernel(x, cond, Wq, Wk, Wv, Wo, bo, _trace=False, _trace_kwargs=None):
    x = np.asarray(x, dtype=np.float32)
    cond = np.asarray(cond, dtype=np.float32)
    nc = _get_nc()
    in_maps = make_in_maps(x, cond,
                           np.asarray(Wq, np.float32), np.asarray(Wk, np.float32),
                           np.asarray(Wv, np.float32), np.asarray(Wo, np.float32))
    kw = {}
    if _trace:
        kw = {"trace": True, "trace_kwargs": _trace_kwargs or {}}
    res = run_bass_kernel_spmd(nc, in_maps, list(range(NCORES)), **kw)
    bo_f = np.asarray(bo, np.float32).reshape(1, D)
    outp = np.empty((B, NQ, D), dtype=np.float32)
    for b in range(B):
        outp[b] = res.results[2 * b]["out"] + res.results[2 * b + 1]["out"] + bo_f
    if _trace:
        return outp, res
    return outp


if __name__ == "__main__":
    # quick numeric self-check against numpy (no jax needed)
    rng = np.random.default_rng(0)
    s = 0.02
    x = rng.standard_normal((B, NQ, D), dtype=np.float32)
    cond = rng.standard_normal((B, NK, D), dtype=np.float32)
    Wq = (rng.standard_normal((D, D), dtype=np.float32) * s)
    Wk = (rng.standard_normal((D, D), dtype=np.float32) * s)
    Wv = (rng.standard_normal((D, D), dtype=np.float32) * s)
    Wo = (rng.standard_normal((D, D), dtype=np.float32) * s)
    bo = (rng.standard_normal((D,), dtype=np.float32) * s)

    def ref_np(x, cond):
        q = (x @ Wq).reshape(B, NQ, H, DH).transpose(0, 2, 1, 3)
        k = (cond @ Wk).reshape(B, NK, H, DH).transpose(0, 2, 1, 3)
        v = (cond @ Wv).reshape(B, NK, H, DH).transpose(0, 2, 1, 3)
        sim = np.einsum('bhid,bhjd->bhij', q, k) * SCALE
        sim = sim - sim.max(axis=-1, keepdims=True)
        a = np.exp(sim)
        a = a / a.sum(axis=-1, keepdims=True)
        o = np.einsum('bhij,bhjd->bhid', a, v)
        o = o.transpose(0, 2, 1, 3).reshape(B, NQ, D)
        return o @ Wo + bo

    import time
    t0 = time.time()
    got = kernel(x=x, cond=cond, Wq=Wq, Wk=Wk, Wv=Wv, Wo=Wo, bo=bo)
    print(f"kernel run {time.time()-t0:.1f}s")
    exp = ref_np(x.astype(np.float64), cond.astype(np.float64))
    err = np.abs(got - exp)
    rel = np.linalg.norm(got - exp) / np.linalg.norm(exp)
    print(f"rel_l2={rel:.3e} absmax_rel={err.max()/np.abs(exp).max():.3e}")


# revision 8
# speedup vs baseline: 1.1184x; 1.1184x over previous
"""CrossAttention TRN2 kernel.

Problem (hardcoded shapes):
  x    [4, 2048, 1024], cond [4, 2048, 1024]
  Wq/Wk/Wv [1024, 1024], Wo [1024, 1024], bo [1024]
  out = softmax((x@Wq) per 8 heads of 128 @ (cond@Wk)^T * 0.125) @ (cond@Wv) @ Wo + bo

Sharding: 8 cores = (batch b in 0..3) x (head-half hh in 0..1).
Each core computes heads hh*4..hh*4+3 over ALL 2048 query rows of one batch.
K/V/Q projections only cover the core's 4 heads (inner cols hh*512..hh*512+512),
so no projection work is replicated. The out-projection contracts only the
core's 512 inner dims, producing a PARTIAL [2048, 1024] output; the host sums
the two partials of each batch and adds bo. No collectives.

All matmul operands are bf16 (fp32 PSUM accumulation): full PE rate, half
LDWEIGHTS/SBUF/DMA cost vs fp32r. Scores stay transposed [j, i] so the
softmax denominator is a partition reduction (Pool engine) of the bf16
exp tiles accumulated on DVE (2x bf16 mode).

Per-head software pipeline: emit K-proj(h+1) after attention(h) so its
matmuls fill the PE stalls left by the exp (Act) dependency; out-proj of
the first i-half starts while the last head's second i-half attention runs.
"""
import numpy as np
import ml_dtypes

import concourse.bass as bass
import concourse.bacc as bacc
import concourse.tile as tile
from concourse import bass_isa, mybir
from concourse.bass_utils import run_bass_kernel_spmd

F32 = mybir.dt.float32
BF16 = mybir.dt.bfloat16
EXP = mybir.ActivationFunctionType.Exp

B, NQ, NK, D = 4, 2048, 2048, 1024   # D = query_dim = cond_dim = inner_dim = out_dim
H, DH = 8, 128                        # heads, per-head dim
HPC = 4                               # heads per core
GW = HPC * DH                         # 512 inner cols per core
SCALE = 64 ** -0.5                    # reference uses dim_head=64 for the scale
NCORES = 8
KT = D // 128                         # contraction tiles (8)
JT = NK // 128                        # key tiles (16)
IC = 1024                             # i-chunk per attention block
NIC = NQ // IC                        # 2


def build_nc():
    nc = bacc.Bacc()
    xT = nc.declare_dram_parameter("xT", [D, NQ], BF16, isOutput=False)
    condT = nc.declare_dram_parameter("condT", [D, NK], BF16, isOutput=False)
    wq = nc.declare_dram_parameter("wq", [D, GW], BF16, isOutput=False)
    wk = nc.declare_dram_parameter("wk", [D, GW], BF16, isOutput=False)
    wv = nc.declare_dram_parameter("wv", [D, GW], BF16, isOutput=False)
    wo = nc.declare_dram_parameter("wo", [GW, D], BF16, isOutput=False)
    out = nc.declare_dram_parameter("out", [NQ, D], F32, isOutput=True)

    with tile.TileContext(nc) as tc:
        with (
            nc.allow_low_precision(reason="bf16 matmul/softmax path is intended"),
            tc.tile_pool(name="wts", bufs=1) as wts,
            tc.tile_pool(name="big", bufs=1) as big,
            tc.tile_pool(name="xstream", bufs=3) as xstream,
            tc.tile_pool(name="expp", bufs=6) as expp,
            tc.tile_pool(name="denp", bufs=2) as denp,
            tc.tile_pool(name="ostage", bufs=2) as ostage,
            tc.tile_pool(name="ps", bufs=1, space="PSUM") as ps,
        ):
            # resident weights
            wq_s = wts.tile([128, KT, GW], BF16, tag="wq_s")
            wk_s = wts.tile([128, KT, GW], BF16, tag="wk_s")
            wv_s = wts.tile([128, KT, GW], BF16, tag="wv_s")
            wo_s = wts.tile([128, HPC, D], BF16, tag="wo_s")
            for k in range(KT):
                rows = slice(k * 128, (k + 1) * 128)
                nc.sync.dma_start(out=wk_s[:, k, :], in_=wk[rows, :])
                nc.sync.dma_start(out=wv_s[:, k, :], in_=wv[rows, :])
                nc.sync.dma_start(out=wq_s[:, k, :], in_=wq[rows, :])
            for h in range(HPC):
                nc.sync.dma_start(out=wo_s[:, h, :],
                                  in_=wo[h * 128:(h + 1) * 128, :])

            # all-ones stationary: den-matmul reduces over partitions AND
            # broadcasts the result to all 128 output partitions in one op
            ones_t = wts.tile([128, 128], BF16, tag="ones_t")
            nc.vector.memset(ones_t, 1.0)

            # resident condT: 8 tiles [128, 2048]
            ct = big.tile([128, KT, NK], BF16, tag="ct")
            for k in range(KT):
                nc.sync.dma_start(out=ct[:, k, :],
                                  in_=condT[k * 128:(k + 1) * 128, :])

            # per-head projections, attention output (all bf16, dh on partitions)
            kT_all = big.tile([128, HPC, NK], BF16, tag="kT_all")
            qT_all = big.tile([128, HPC, NQ], BF16, tag="qT_all")
            v_all = big.tile([128, JT, GW], BF16, tag="v_all")
            attT = big.tile([128, HPC, NQ], BF16, tag="attT")

            def k_proj(h):
                # kT_all[:, h, :] = (Wk_h)^T @ condT ; [dh=128, j=2048]
                for jp in range(NK // 1024):
                    acc = ps.tile([128, 1024], F32, tag="sc", bufs=2,
                                  name=f"kacc_{h}_{jp}")
                    for half in range(2):
                        jh = jp * 2 + half
                        for k in range(KT):
                            nc.tensor.matmul(
                                acc[:, half * 512:(half + 1) * 512],
                                wk_s[:, k, h * DH:(h + 1) * DH],
                                ct[:, k, jh * 512:(jh + 1) * 512],
                                start=(k == 0), stop=(k == KT - 1))
                    nc.vector.tensor_copy(
                        kT_all[:, h, jp * 1024:(jp + 1) * 1024], acc)

            def q_proj_all():
                # qT_all[:, h, :] = (Wq_h)^T @ xT for all 4 heads, x streamed once
                for ih in range(NQ // 512):
                    accs = [ps.tile([128, 1024], F32, tag="sc", bufs=2,
                                    name=f"qacc_{ih}_{hp}") for hp in range(2)]
                    xk_tiles = []
                    for k in range(KT):
                        xk = xstream.tile([128, 512], BF16, tag="xk",
                                          name=f"xk_{ih}_{k}")
                        nc.sync.dma_start(
                            out=xk,
                            in_=xT[k * 128:(k + 1) * 128, ih * 512:(ih + 1) * 512])
                        xk_tiles.append(xk)
                        for h in range(HPC):
                            nc.tensor.matmul(
                                accs[h // 2][:, (h % 2) * 512:(h % 2) * 512 + 512],
                                wq_s[:, k, h * DH:(h + 1) * DH],
                                xk,
                                start=(k == 0), stop=(k == KT - 1))
                    for h in range(HPC):
                        nc.vector.tensor_copy(
                            qT_all[:, h, ih * 512:(ih + 1) * 512],
                            accs[h // 2][:, (h % 2) * 512:(h % 2) * 512 + 512])

            def v_proj_all():
                # v_all[:, jt, :] = condT_jt^T @ Wv (all 4 heads); [j=128, 512]
                for jp in range(JT // 2):
                    acc = ps.tile([128, 1024], F32, tag="av", bufs=2,
                                  name=f"vacc_{jp}")
                    for half in range(2):
                        jt = jp * 2 + half
                        for k in range(KT):
                            nc.tensor.matmul(
                                acc[:, half * 512:(half + 1) * 512],
                                ct[:, k, jt * 128:(jt + 1) * 128],
                                wv_s[:, k, :],
                                start=(k == 0), stop=(k == KT - 1))
                    for half in range(2):
                        nc.vector.tensor_copy(
                            v_all[:, jp * 2 + half, :],
                            acc[:, half * 512:(half + 1) * 512])

            def attention(h):
                for ic in range(NIC):
                    i0 = ic * IC
                    avs = ps.tile([128, IC], F32, tag="av", bufs=2,
                                  name=f"avs_{h}_{ic}")
                    den_s = denp.tile([128, IC], BF16, tag="den_s",
                                      name=f"den_s_{h}_{ic}")
                    for jt in range(JT):
                        sc = ps.tile([128, IC], F32, tag="sc", bufs=2)
                        for ih in range(IC // 512):
                            nc.tensor.matmul(
                                sc[:, ih * 512:(ih + 1) * 512],
                                kT_all[:, h, jt * 128:(jt + 1) * 128],
                                qT_all[:, h, i0 + ih * 512:i0 + (ih + 1) * 512],
                                start=True, stop=True)
                        esc = expp.tile([128, IC], BF16, tag="esc")
                        nc.scalar.activation(esc, sc, EXP)
                        for ih in range(IC // 512):
                            nc.tensor.matmul(
                                avs[:, ih * 512:(ih + 1) * 512],
                                v_all[:, jt, h * DH:(h + 1) * DH],
                                esc[:, ih * 512:(ih + 1) * 512],
                                start=(jt == 0), stop=(jt == JT - 1))
                        if jt == 0:
                            nc.vector.tensor_copy(den_s, esc)
                        else:
                            nc.vector.tensor_add(den_s, den_s, esc)
                    dsum = ps.tile([128, IC], F32, tag="sc", bufs=2,
                                   name=f"dsum_{h}_{ic}")
                    for ih in range(IC // 512):
                        nc.tensor.matmul(
                            dsum[:, ih * 512:(ih + 1) * 512],
                            ones_t,
                            den_s[:, ih * 512:(ih + 1) * 512],
                            start=True, stop=True)
                    den_bc = denp.tile([128, IC], F32, tag="den_bc",
                                       name=f"den_bc_{h}_{ic}")
                    nc.vector.reciprocal_approx_fast(den_bc, dsum)
                    nc.vector.tensor_mul(
                        attT[:, h, i0:i0 + IC], avs, den_bc)

            def out_proj(it):
                # out[it*128:(it+1)*128, :] = attT_it^T @ Wo  (partial; host adds bo)
                ot = ps.tile([128, D], F32, tag="av", bufs=2, name=f"ot_{it}")
                for nh in range(D // 512):
                    for h in range(HPC):
                        nc.tensor.matmul(
                            ot[:, nh * 512:(nh + 1) * 512],
                            attT[:, h, it * 128:(it + 1) * 128],
                            wo_s[:, h, nh * 512:(nh + 1) * 512],
                            start=(h == 0), stop=(h == HPC - 1))
                fo = ostage.tile([128, D], F32, tag="fo")
                nc.vector.tensor_copy(fo, ot)
                nc.sync.dma_start(
                    out=out[it * 128:(it + 1) * 128, :], in_=fo)

            # emission order = scheduler priority: attention(h) before
            # k_proj(h+1) so projection matmuls fill attention's Act stalls.
            k_proj(0)
            v_proj_all()
            q_proj_all()
            attention(0)
            k_proj(1)
            attention(1)
            k_proj(2)
            attention(2)
            k_proj(3)
            attention(3)
            for it in range(NQ // 128):
                out_proj(it)
    nc.finalize()
    return nc


_NC_CACHE = None


def _get_nc():
    global _NC_CACHE
    if _NC_CACHE is None:
        _NC_CACHE = build_nc()
    return _NC_CACHE


def make_in_maps(x, cond, Wq, Wk, Wv, Wo):
    bf = ml_dtypes.bfloat16
    wq_s = (Wq.astype(np.float64) * SCALE).astype(bf)
    wk_c = Wk.astype(bf)
    wv_c = Wv.astype(bf)
    wo_c = Wo.astype(bf)
    in_maps = []
    for c in range(NCORES):
        b, hh = c // 2, c % 2
        cols = slice(hh * GW, (hh + 1) * GW)
        in_maps.append({
            "xT": np.ascontiguousarray(x[b].T.astype(bf)),
            "condT": np.ascontiguousarray(cond[b].T.astype(bf)),
            "wq": np.ascontiguousarray(wq_s[:, cols]),
            "wk": np.ascontiguousarray(wk_c[:, cols]),
            "wv": np.ascontiguousarray(wv_c[:, cols]),
            "wo": np.ascontiguousarray(wo_c[cols, :]),
        })
    return in_maps


def kernel(x, cond, Wq, Wk, Wv, Wo, bo, _trace=False, _trace_kwargs=None):
    x = np.asarray(x, dtype=np.float32)
    cond = np.asarray(cond, dtype=np.float32)
    nc = _get_nc()
    in_maps = make_in_maps(x, cond,
                           np.asarray(Wq, np.float32), np.asarray(Wk, np.float32),
                           np.asarray(Wv, np.float32), np.asarray(Wo, np.float32))
    kw = {}
    if _trace:
        kw = {"trace": True, "trace_kwargs": _trace_kwargs or {}}
    res = run_bass_kernel_spmd(nc, in_maps, list(range(NCORES)), **kw)
    bo_f = np.asarray(bo, np.float32).reshape(1, D)
    outp = np.empty((B, NQ, D), dtype=np.float32)
    for b in range(B):
        outp[b] = res.results[2 * b]["out"] + res.results[2 * b + 1]["out"] + bo_f
    if _trace:
        return outp, res
    return outp


if __name__ == "__main__":
    # quick numeric self-check against numpy (no jax needed)
    rng = np.random.default_rng(0)
    s = 0.02
    x = rng.standard_normal((B, NQ, D), dtype=np.float32)
    cond = rng.standard_normal((B, NK, D), dtype=np.float32)
    Wq = (rng.standard_normal((D, D), dtype=np.float32) * s)
    Wk = (rng.standard_normal((D, D), dtype=np.float32) * s)
    Wv = (rng.standard_normal((D, D), dtype=np.float32) * s)
    Wo = (rng.standard_normal((D, D), dtype=np.float32) * s)
    bo = (rng.standard_normal((D,), dtype=np.float32) * s)

    def ref_np(x, cond):
        q = (x @ Wq).reshape(B, NQ, H, DH).transpose(0, 2, 1, 3)
        k = (cond @ Wk).reshape(B, NK, H, DH).transpose(0, 2, 1, 3)
        v = (cond @ Wv).reshape(B, NK, H, DH).transpose(0, 2, 1, 3)
        sim = np.einsum('bhid,bhjd->bhij', q, k) * SCALE
        sim = sim - sim.max(axis=-1, keepdims=True)
        a = np.exp(sim)
        a = a / a.sum(axis=-1, keepdims=True)
        o = np.einsum('bhij,bhjd->bhid', a, v)
        o = o.transpose(0, 2, 1, 3).reshape(B, NQ, D)
        return o @ Wo + bo

    import time
    t0 = time.time()
    got = kernel(x=x, cond=cond, Wq=Wq, Wk=Wk, Wv=Wv, Wo=Wo, bo=bo)
    print(f"kernel run {time.time()-t0:.1f}s")
    exp = ref_np(x.astype(np.float64), cond.astype(np.float64))
    err = np.abs(got - exp)
    rel = np.linalg.norm(got - exp) / np.linalg.norm(exp)
    print(f"rel_l2={rel:.3e} absmax_rel={err.max()/np.abs(exp).max():.3e}")


# revision 15
# speedup vs baseline: 1.1962x; 1.0695x over previous
"""CrossAttention TRN2 kernel.

Problem (hardcoded shapes):
  x    [4, 2048, 1024], cond [4, 2048, 1024]
  Wq/Wk/Wv [1024, 1024], Wo [1024, 1024], bo [1024]
  out = softmax((x@Wq) per 8 heads of 128 @ (cond@Wk)^T * 0.125) @ (cond@Wv) @ Wo + bo

Sharding: 8 cores = (batch b in 0..3) x (head-half hh in 0..1).
Each core computes heads hh*4..hh*4+3 over ALL 2048 query rows of one batch.
K/V/Q projections only cover the core's 4 heads (inner cols hh*512..hh*512+512),
so no projection work is replicated. The out-projection contracts only the
core's 512 inner dims, producing a PARTIAL [2048, 1024] output; the host sums
the two partials of each batch and adds bo. No collectives.

All matmul operands are bf16 (fp32 PSUM accumulation): full PE rate, half
LDWEIGHTS/SBUF/DMA cost vs fp32r. Scores stay transposed [j, i] so the
softmax denominator is a partition reduction (Pool engine) of the bf16
exp tiles accumulated on DVE (2x bf16 mode).

Per-head software pipeline: emit K-proj(h+1) after attention(h) so its
matmuls fill the PE stalls left by the exp (Act) dependency; out-proj of
the first i-half starts while the last head's second i-half attention runs.
"""
import numpy as np
import ml_dtypes

import concourse.bass as bass
import concourse.bacc as bacc
import concourse.tile as tile
from concourse import bass_isa, mybir
from concourse.bass_utils import run_bass_kernel_spmd

F32 = mybir.dt.float32
BF16 = mybir.dt.bfloat16
EXP = mybir.ActivationFunctionType.Exp

B, NQ, NK, D = 4, 2048, 2048, 1024   # D = query_dim = cond_dim = inner_dim = out_dim
H, DH = 8, 128                        # heads, per-head dim
HPC = 4                               # heads per core
GW = HPC * DH                         # 512 inner cols per core
SCALE = 64 ** -0.5                    # reference uses dim_head=64 for the scale
NCORES = 8
KT = D // 128                         # contraction tiles (8)
JT = NK // 128                        # key tiles (16)
IC = 1024                             # i-chunk per attention block
NIC = NQ // IC                        # 2


def build_nc():
    nc = bacc.Bacc()
    xT = nc.declare_dram_parameter("xT", [D, NQ], BF16, isOutput=False)
    condT = nc.declare_dram_parameter("condT", [D, NK], BF16, isOutput=False)
    wq = nc.declare_dram_parameter("wq", [D, GW], BF16, isOutput=False)
    wk = nc.declare_dram_parameter("wk", [D, GW], BF16, isOutput=False)
    wv = nc.declare_dram_parameter("wv", [D, GW], BF16, isOutput=False)
    wo = nc.declare_dram_parameter("wo", [GW, D], BF16, isOutput=False)
    out = nc.declare_dram_parameter("out", [NQ, D], F32, isOutput=True)

    with tile.TileContext(nc) as tc:
        with (
            nc.allow_low_precision(reason="bf16 matmul/softmax path is intended"),
            tc.tile_pool(name="wts", bufs=1) as wts,
            tc.tile_pool(name="big", bufs=1) as big,
            tc.tile_pool(name="xstream", bufs=3) as xstream,  # [128,KT,512] tiles
            tc.tile_pool(name="expp", bufs=6) as expp,
            tc.tile_pool(name="denp", bufs=2) as denp,
            tc.tile_pool(name="ostage", bufs=2) as ostage,
            tc.tile_pool(name="ps", bufs=1, space="PSUM") as ps,
        ):
            # resident weights
            wq_s = wts.tile([128, KT, GW], BF16, tag="wq_s")
            wk_s = wts.tile([128, KT, GW], BF16, tag="wk_s")
            wv_s = wts.tile([128, KT, GW], BF16, tag="wv_s")
            wo_s = wts.tile([128, HPC, D], BF16, tag="wo_s")
            for k in range(KT):
                rows = slice(k * 128, (k + 1) * 128)
                nc.sync.dma_start(out=wk_s[:, k, :], in_=wk[rows, :])
                nc.sync.dma_start(out=wv_s[:, k, :], in_=wv[rows, :])
                nc.sync.dma_start(out=wq_s[:, k, :], in_=wq[rows, :])
            for h in range(HPC):
                nc.sync.dma_start(out=wo_s[:, h, :],
                                  in_=wo[h * 128:(h + 1) * 128, :])

            # all-ones stationary: den-matmul reduces over partitions AND
            # broadcasts the result to all 128 output partitions in one op
            ones_t = wts.tile([128, 128], BF16, tag="ones_t")
            nc.vector.memset(ones_t, 1.0)

            # resident condT: 8 tiles [128, 2048]
            ct = big.tile([128, KT, NK], BF16, tag="ct")
            for k in range(KT):
                nc.sync.dma_start(out=ct[:, k, :],
                                  in_=condT[k * 128:(k + 1) * 128, :])

            # per-head projections, attention output (all bf16, dh on partitions)
            kT_all = big.tile([128, HPC, NK], BF16, tag="kT_all")
            qT_all = big.tile([128, HPC, NQ], BF16, tag="qT_all")
            v_all = big.tile([128, JT, GW], BF16, tag="v_all")
            attT = big.tile([128, HPC, NQ], BF16, tag="attT")

            def k_proj(h, tag="fill", bufs=1):
                # kT_all[:, h, :] = (Wk_h)^T @ condT ; [dh=128, j=2048]
                for jp in range(NK // 1024):
                    acc = ps.tile([128, 1024], F32, tag=tag, bufs=bufs,
                                  name=f"kacc_{h}_{jp}")
                    for half in range(2):
                        jh = jp * 2 + half
                        for k in range(KT):
                            nc.tensor.matmul(
                                acc[:, half * 512:(half + 1) * 512],
                                wk_s[:, k, h * DH:(h + 1) * DH],
                                ct[:, k, jh * 512:(jh + 1) * 512],
                                start=(k == 0), stop=(k == KT - 1))
                    nc.vector.tensor_copy(
                        kT_all[:, h, jp * 1024:(jp + 1) * 1024], acc)

            def q_proj_all():
                # qT_all[:, h, :] = (Wq_h)^T @ xT for all 4 heads, x streamed once
                for ih in range(NQ // 512):
                    xk = xstream.tile([128, KT, 512], BF16, tag="xk",
                                      name=f"xk_{ih}")
                    for k in range(KT):
                        nc.sync.dma_start(
                            out=xk[:, k, :],
                            in_=xT[k * 128:(k + 1) * 128, ih * 512:(ih + 1) * 512])
                    for hp in range(2):
                        acc = ps.tile([128, 1024], F32, tag="sc", bufs=2,
                                      name=f"qacc_{ih}_{hp}")
                        for k in range(KT):
                            for hh in range(2):
                                h = hp * 2 + hh
                                nc.tensor.matmul(
                                    acc[:, hh * 512:(hh + 1) * 512],
                                    wq_s[:, k, h * DH:(h + 1) * DH],
                                    xk[:, k, :],
                                    start=(k == 0), stop=(k == KT - 1))
                        for hh in range(2):
                            h = hp * 2 + hh
                            nc.vector.tensor_copy(
                                qT_all[:, h, ih * 512:(ih + 1) * 512],
                                acc[:, hh * 512:(hh + 1) * 512])

            def v_proj_all():
                # v_all[:, jt, :] = condT_jt^T @ Wv (all 4 heads); [j=128, 512]
                for jp in range(JT // 2):
                    acc = ps.tile([128, 1024], F32, tag="sc", bufs=2,
                                  name=f"vacc_{jp}")
                    for half in range(2):
                        jt = jp * 2 + half
                        for k in range(KT):
                            nc.tensor.matmul(
                                acc[:, half * 512:(half + 1) * 512],
                                ct[:, k, jt * 128:(jt + 1) * 128],
                                wv_s[:, k, :],
                                start=(k == 0), stop=(k == KT - 1))
                    for half in range(2):
                        nc.vector.tensor_copy(
                            v_all[:, jp * 2 + half, :],
                            acc[:, half * 512:(half + 1) * 512])

            def attention(h):
                for ic in range(NIC):
                    i0 = ic * IC
                    avs = ps.tile([128, IC], F32, tag="av", bufs=1,
                                  name=f"avs_{h}_{ic}")
                    den_s = denp.tile([128, IC], BF16, tag="den_s",
                                      name=f"den_s_{h}_{ic}")
                    for jt in range(JT):
                        sc = ps.tile([128, IC], F32, tag="sc", bufs=2)
                        for ih in range(IC // 512):
                            nc.tensor.matmul(
                                sc[:, ih * 512:(ih + 1) * 512],
                                kT_all[:, h, jt * 128:(jt + 1) * 128],
                                qT_all[:, h, i0 + ih * 512:i0 + (ih + 1) * 512],
                                start=True, stop=True)
                        esc = expp.tile([128, IC], BF16, tag="esc")
                        nc.scalar.activation(esc, sc, EXP)
                        for ih in range(IC // 512):
                            nc.tensor.matmul(
                                avs[:, ih * 512:(ih + 1) * 512],
                                v_all[:, jt, h * DH:(h + 1) * DH],
                                esc[:, ih * 512:(ih + 1) * 512],
                                start=(jt == 0), stop=(jt == JT - 1))
                        if jt == 0:
                            nc.vector.tensor_copy(den_s, esc)
                        else:
                            nc.vector.tensor_add(den_s, den_s, esc)
                    dsum = ps.tile([128, IC], F32, tag="sc", bufs=2,
                                   name=f"dsum_{h}_{ic}")
                    for ih in range(IC // 512):
                        nc.tensor.matmul(
                            dsum[:, ih * 512:(ih + 1) * 512],
                            ones_t,
                            den_s[:, ih * 512:(ih + 1) * 512],
                            start=True, stop=True)
                    den_bc = denp.tile([128, IC], F32, tag="den_bc",
                                       name=f"den_bc_{h}_{ic}")
                    nc.vector.reciprocal_approx_fast(den_bc, dsum)
                    nc.vector.tensor_mul(
                        attT[:, h, i0:i0 + IC], avs, den_bc)

            def out_proj(it):
                # out[it*128:(it+1)*128, :] = attT_it^T @ Wo  (partial; host adds bo)
                ot = ps.tile([128, D], F32, tag="fill", bufs=1, name=f"ot_{it}")
                for nh in range(D // 512):
                    for h in range(HPC):
                        nc.tensor.matmul(
                            ot[:, nh * 512:(nh + 1) * 512],
                            attT[:, h, it * 128:(it + 1) * 128],
                            wo_s[:, h, nh * 512:(nh + 1) * 512],
                            start=(h == 0), stop=(h == HPC - 1))
                fo = ostage.tile([128, D], F32, tag="fo")
                nc.vector.tensor_copy(fo, ot)
                nc.sync.dma_start(
                    out=out[it * 128:(it + 1) * 128, :], in_=fo)

            # emission order = scheduler priority: attention(h) before
            # k_proj(h+1) so projection matmuls fill attention's Act stalls.
            k_proj(0, tag="sc", bufs=2)
            v_proj_all()
            q_proj_all()
            attention(0)
            k_proj(1)
            attention(1)
            k_proj(2)
            attention(2)
            k_proj(3)
            attention(3)
            for it in range(NQ // 128):
                out_proj(it)
    nc.finalize()
    return nc


_NC_CACHE = None


def _get_nc():
    global _NC_CACHE
    if _NC_CACHE is None:
        _NC_CACHE = build_nc()
    return _NC_CACHE


def make_in_maps(x, cond, Wq, Wk, Wv, Wo):
    bf = ml_dtypes.bfloat16
    wq_s = (Wq.astype(np.float64) * SCALE).astype(bf)
    wk_c = Wk.astype(bf)
    wv_c = Wv.astype(bf)
    wo_c = Wo.astype(bf)
    in_maps = []
    for c in range(NCORES):
        b, hh = c // 2, c % 2
        cols = slice(hh * GW, (hh + 1) * GW)
        in_maps.append({
            "xT": np.ascontiguousarray(x[b].T.astype(bf)),
            "condT": np.ascontiguousarray(cond[b].T.astype(bf)),
            "wq": np.ascontiguousarray(wq_s[:, cols]),
            "wk": np.ascontiguousarray(wk_c[:, cols]),
            "wv": np.ascontiguousarray(wv_c[:, cols]),
            "wo": np.ascontiguousarray(wo_c[cols, :]),
        })
    return in_maps


def kernel(x, cond, Wq, Wk, Wv, Wo, bo, _trace=False, _trace_kwargs=None):
    x = np.asarray(x, dtype=np.float32)
    cond = np.asarray(cond, dtype=np.float32)
    nc = _get_nc()
    in_maps = make_in_maps(x, cond,
                           np.asarray(Wq, np.float32), np.asarray(Wk, np.float32),
                           np.asarray(Wv, np.float32), np.asarray(Wo, np.float32))
    kw = {}
    if _trace:
        kw = {"trace": True, "trace_kwargs": _trace_kwargs or {}}
    res = run_bass_kernel_spmd(nc, in_maps, list(range(NCORES)), **kw)
    bo_f = np.asarray(bo, np.float32).reshape(1, D)
    outp = np.empty((B, NQ, D), dtype=np.float32)
    for b in range(B):
        outp[b] = res.results[2 * b]["out"] + res.results[2 * b + 1]["out"] + bo_f
    if _trace:
        return outp, res
    return outp


if __name__ == "__main__":
    # quick numeric self-check against numpy (no jax needed)
    rng = np.random.default_rng(0)
    s = 0.02
    x = rng.standard_normal((B, NQ, D), dtype=np.float32)
    cond = rng.standard_normal((B, NK, D), dtype=np.float32)
    Wq = (rng.standard_normal((D, D), dtype=np.float32) * s)
    Wk = (rng.standard_normal((D, D), dtype=np.float32) * s)
    Wv = (rng.standard_normal((D, D), dtype=np.float32) * s)
    Wo = (rng.standard_normal((D, D), dtype=np.float32) * s)
    bo = (rng.standard_normal((D,), dtype=np.float32) * s)

    def ref_np(x, cond):
        q = (x @ Wq).reshape(B, NQ, H, DH).transpose(0, 2, 1, 3)
        k = (cond @ Wk).reshape(B, NK, H, DH).transpose(0, 2, 1, 3)
        v = (cond @ Wv).reshape(B, NK, H, DH).transpose(0, 2, 1, 3)
        sim = np.einsum('bhid,bhjd->bhij', q, k) * SCALE
        sim = sim - sim.max(axis=-1, keepdims=True)
        a = np.exp(sim)
        a = a / a.sum(axis=-1, keepdims=True)
        o = np.einsum('bhij,bhjd->bhid', a, v)
        o = o.transpose(0, 2, 1, 3).reshape(B, NQ, D)
        return o @ Wo + bo

    import time
    t0 = time.time()
    got = kernel(x=x, cond=cond, Wq=Wq, Wk=Wk, Wv=Wv, Wo=Wo, bo=bo)
    print(f"kernel run {time.time()-t0:.1f}s")
    exp = ref_np(x.astype(np.float64), cond.astype(np.float64))
    err = np.abs(got - exp)
    rel = np.linalg.norm(got - exp) / np.linalg.norm(exp)
    print(f"rel_l2={rel:.3e} absmax_rel={err.max()/np.abs(exp).max():.3e}")


# revision 21
# speedup vs baseline: 1.2914x; 1.0796x over previous
"""CrossAttention TRN2 kernel.

Problem (hardcoded shapes):
  x    [4, 2048, 1024], cond [4, 2048, 1024]
  Wq/Wk/Wv [1024, 1024], Wo [1024, 1024], bo [1024]
  out = softmax((x@Wq) per 8 heads of 128 @ (cond@Wk)^T * 0.125) @ (cond@Wv) @ Wo + bo

Sharding: 8 cores = (batch b in 0..3) x (head-half hh in 0..1).
Each core computes heads hh*4..hh*4+3 over ALL 2048 query rows of one batch.
K/V/Q projections only cover the core's 4 heads (inner cols hh*512..hh*512+512),
so no projection work is replicated. The out-projection contracts only the
core's 512 inner dims, producing a PARTIAL [2048, 1024] output; the host sums
the two partials of each batch and adds bo. No collectives.

All matmul operands are bf16 (fp32 PSUM accumulation): full PE rate, half
LDWEIGHTS/SBUF/DMA cost vs fp32r. Scores stay transposed [j, i] so the
softmax denominator is a partition reduction (Pool engine) of the bf16
exp tiles accumulated on DVE (2x bf16 mode).

Per-head software pipeline: emit K-proj(h+1) after attention(h) so its
matmuls fill the PE stalls left by the exp (Act) dependency; out-proj of
the first i-half starts while the last head's second i-half attention runs.
"""
import numpy as np
import ml_dtypes

import concourse.bass as bass
import concourse.bacc as bacc
import concourse.tile as tile
from concourse import bass_isa, mybir
from concourse.bass_utils import run_bass_kernel_spmd

F32 = mybir.dt.float32
BF16 = mybir.dt.bfloat16
EXP = mybir.ActivationFunctionType.Exp

B, NQ, NK, D = 4, 2048, 2048, 1024   # D = query_dim = cond_dim = inner_dim = out_dim
H, DH = 8, 128                        # heads, per-head dim
HPC = 4                               # heads per core
GW = HPC * DH                         # 512 inner cols per core
SCALE = 64 ** -0.5                    # reference uses dim_head=64 for the scale
NCORES = 8
KT = D // 128                         # contraction tiles (8)
JT = NK // 128                        # key tiles (16)
IC = 1024                             # i-chunk per attention block
NIC = NQ // IC                        # 2


def build_nc():
    nc = bacc.Bacc()
    xT = nc.declare_dram_parameter("xT", [D, NQ], BF16, isOutput=False)
    condT = nc.declare_dram_parameter("condT", [D, NK], BF16, isOutput=False)
    wq = nc.declare_dram_parameter("wq", [D, GW], BF16, isOutput=False)
    wk = nc.declare_dram_parameter("wk", [D, GW], BF16, isOutput=False)
    wv = nc.declare_dram_parameter("wv", [D, GW], BF16, isOutput=False)
    wo = nc.declare_dram_parameter("wo", [GW, D], BF16, isOutput=False)
    out = nc.declare_dram_parameter("out", [NQ, D], F32, isOutput=True)

    with tile.TileContext(nc) as tc:
        with (
            nc.allow_low_precision(reason="bf16 matmul/softmax path is intended"),
            tc.tile_pool(name="wts", bufs=1) as wts,
            tc.tile_pool(name="big", bufs=1) as big,
            tc.tile_pool(name="xstream", bufs=3) as xstream,  # [128,KT,512] tiles
            tc.tile_pool(name="expp", bufs=8) as expp,
            tc.tile_pool(name="denp", bufs=2) as denp,
            tc.tile_pool(name="ostage", bufs=2) as ostage,
            tc.tile_pool(name="ps", bufs=1, space="PSUM") as ps,
        ):
            # resident weights + condT, DMA'd in consumption order:
            # k_proj(0) streams (wk[k], ct[k]) pairs, then v needs wv,
            # then q needs wq; wo only at the very end.
            wq_s = wts.tile([128, KT, GW], BF16, tag="wq_s")
            wk_s = wts.tile([128, KT, GW], BF16, tag="wk_s")
            wv_s = wts.tile([128, KT, GW], BF16, tag="wv_s")
            wo_s = wts.tile([128, HPC, D], BF16, tag="wo_s")
            ct = big.tile([128, KT, NK], BF16, tag="ct")
            for k in range(KT):
                rows = slice(k * 128, (k + 1) * 128)
                nc.sync.dma_start(out=wk_s[:, k, :], in_=wk[rows, :])
                nc.sync.dma_start(out=ct[:, k, :], in_=condT[rows, :])
            for k in range(KT):
                rows = slice(k * 128, (k + 1) * 128)
                nc.sync.dma_start(out=wv_s[:, k, :], in_=wv[rows, :])
            for k in range(KT):
                rows = slice(k * 128, (k + 1) * 128)
                nc.sync.dma_start(out=wq_s[:, k, :], in_=wq[rows, :])
            for h in range(HPC):
                nc.sync.dma_start(out=wo_s[:, h, :],
                                  in_=wo[h * 128:(h + 1) * 128, :])

            # all-ones stationary: den-matmul reduces over partitions AND
            # broadcasts the result to all 128 output partitions in one op
            ones_t = wts.tile([128, 128], BF16, tag="ones_t")
            nc.vector.memset(ones_t, 1.0)

            # per-head projections, attention output (all bf16, dh on partitions)
            kT_all = big.tile([128, HPC, NK], BF16, tag="kT_all")
            qT_all = big.tile([128, HPC, NQ], BF16, tag="qT_all")
            v_all = big.tile([128, JT, GW], BF16, tag="v_all")
            attT = big.tile([128, HPC, NQ], BF16, tag="attT")

            def k_proj(h):
                # kT_all[:, h, :] = (Wk_h)^T @ condT ; [dh=128, j=2048]
                # [128,512] accs through the depth-2 fill ring: next acc's
                # matmuls overlap the previous acc's copy-out.
                for jh in range(NK // 512):
                    acc = ps.tile([128, 512], F32, tag="fill", bufs=2,
                                  name=f"kacc_{h}_{jh}")
                    for k in range(KT):
                        nc.tensor.matmul(
                            acc,
                            wk_s[:, k, h * DH:(h + 1) * DH],
                            ct[:, k, jh * 512:(jh + 1) * 512],
                            start=(k == 0), stop=(k == KT - 1))
                    nc.vector.tensor_copy(
                        kT_all[:, h, jh * 512:(jh + 1) * 512], acc)

            def q_proj_heads(hp):
                # qT_all for heads {2hp, 2hp+1}; streams xT (re-read per hp)
                for ih in range(NQ // 512):
                    xk = xstream.tile([128, KT, 512], BF16, tag="xk",
                                      name=f"xk_{hp}_{ih}")
                    for k in range(KT):
                        nc.sync.dma_start(
                            out=xk[:, k, :],
                            in_=xT[k * 128:(k + 1) * 128, ih * 512:(ih + 1) * 512])
                    for hh in range(2):
                        h = hp * 2 + hh
                        acc = ps.tile([128, 512], F32, tag="fill", bufs=2,
                                      name=f"qacc_{hp}_{ih}_{hh}")
                        for k in range(KT):
                            nc.tensor.matmul(
                                acc,
                                wq_s[:, k, h * DH:(h + 1) * DH],
                                xk[:, k, :],
                                start=(k == 0), stop=(k == KT - 1))
                        nc.vector.tensor_copy(
                            qT_all[:, h, ih * 512:(ih + 1) * 512], acc)

            def v_proj_all():
                # v_all[:, jt, :] = condT_jt^T @ Wv (all 4 heads); [j=128, 512]
                for jp in range(JT // 2):
                    acc = ps.tile([128, 1024], F32, tag="sc", bufs=2,
                                  name=f"vacc_{jp}")
                    for half in range(2):
                        jt = jp * 2 + half
                        for k in range(KT):
                            nc.tensor.matmul(
                                acc[:, half * 512:(half + 1) * 512],
                                ct[:, k, jt * 128:(jt + 1) * 128],
                                wv_s[:, k, :],
                                start=(k == 0), stop=(k == KT - 1))
                    for half in range(2):
                        nc.vector.tensor_copy(
                            v_all[:, jp * 2 + half, :],
                            acc[:, half * 512:(half + 1) * 512])

            def attention(h):
                for ic in range(NIC):
                    i0 = ic * IC
                    avs = ps.tile([128, IC], F32, tag="av", bufs=1,
                                  name=f"avs_{h}_{ic}")
                    den_s = denp.tile([128, IC], BF16, tag="den_s",
                                      name=f"den_s_{h}_{ic}")
                    for jt in range(JT):
                        sc = ps.tile([128, IC], F32, tag="sc", bufs=2)
                        for ih in range(IC // 512):
                            nc.tensor.matmul(
                                sc[:, ih * 512:(ih + 1) * 512],
                                kT_all[:, h, jt * 128:(jt + 1) * 128],
                                qT_all[:, h, i0 + ih * 512:i0 + (ih + 1) * 512],
                                start=True, stop=True)
                        esc = expp.tile([128, IC], BF16, tag="esc")
                        nc.scalar.activation(esc, sc, EXP)
                        for ih in range(IC // 512):
                            nc.tensor.matmul(
                                avs[:, ih * 512:(ih + 1) * 512],
                                v_all[:, jt, h * DH:(h + 1) * DH],
                                esc[:, ih * 512:(ih + 1) * 512],
                                start=(jt == 0), stop=(jt == JT - 1))
                        if jt == 0:
                            nc.vector.tensor_copy(den_s, esc)
                        else:
                            nc.vector.tensor_add(den_s, den_s, esc)
                    dsum = ps.tile([128, IC], F32, tag="sc", bufs=2,
                                   name=f"dsum_{h}_{ic}")
                    for ih in range(IC // 512):
                        nc.tensor.matmul(
                            dsum[:, ih * 512:(ih + 1) * 512],
                            ones_t,
                            den_s[:, ih * 512:(ih + 1) * 512],
                            start=True, stop=True)
                    den_bc = denp.tile([128, IC], F32, tag="den_bc",
                                       name=f"den_bc_{h}_{ic}")
                    nc.vector.reciprocal_approx_fast(den_bc, dsum)
                    nc.vector.tensor_mul(
                        attT[:, h, i0:i0 + IC], avs, den_bc)

            def out_proj(it):
                # out[it*128:(it+1)*128, :] = attT_it^T @ Wo  (partial; host adds bo)
                # even it: two [128,512] tiles through the fill ring (runs as
                # attention filler); odd it: one [128,1024] tile on the av ring
                # (free once the final avs has drained).
                fo = ostage.tile([128, D], F32, tag="fo")
                if it % 2 == 0:
                    for nh in range(D // 512):
                        ot = ps.tile([128, 512], F32, tag="fill", bufs=2,
                                     name=f"ot_{it}_{nh}")
                        for h in range(HPC):
                            nc.tensor.matmul(
                                ot,
                                attT[:, h, it * 128:(it + 1) * 128],
                                wo_s[:, h, nh * 512:(nh + 1) * 512],
                                start=(h == 0), stop=(h == HPC - 1))
                        nc.vector.tensor_copy(
                            fo[:, nh * 512:(nh + 1) * 512], ot)
                else:
                    ot = ps.tile([128, D], F32, tag="av", bufs=1,
                                 name=f"ot_{it}")
                    for nh in range(D // 512):
                        for h in range(HPC):
                            nc.tensor.matmul(
                                ot[:, nh * 512:(nh + 1) * 512],
                                attT[:, h, it * 128:(it + 1) * 128],
                                wo_s[:, h, nh * 512:(nh + 1) * 512],
                                start=(h == 0), stop=(h == HPC - 1))
                    nc.vector.tensor_copy(fo, ot)
                nc.sync.dma_start(
                    out=out[it * 128:(it + 1) * 128, :], in_=fo)

            # emission order = scheduler priority: attention(h) emitted before
            # later projections so those matmuls fill attention's Act stalls.
            k_proj(0)
            q_proj_heads(0)
            v_proj_all()
            attention(0)
            k_proj(1)
            q_proj_heads(1)
            attention(1)
            k_proj(2)
            attention(2)
            k_proj(3)
            attention(3)
            for it in range(NQ // 128):
                out_proj(it)
    nc.finalize()
    return nc


_NC_CACHE = None


def _get_nc():
    global _NC_CACHE
    if _NC_CACHE is None:
        _NC_CACHE = build_nc()
    return _NC_CACHE


def make_in_maps(x, cond, Wq, Wk, Wv, Wo):
    bf = ml_dtypes.bfloat16
    wq_s = (Wq.astype(np.float64) * SCALE).astype(bf)
    wk_c = Wk.astype(bf)
    wv_c = Wv.astype(bf)
    wo_c = Wo.astype(bf)
    in_maps = []
    for c in range(NCORES):
        b, hh = c // 2, c % 2
        cols = slice(hh * GW, (hh + 1) * GW)
        in_maps.append({
            "xT": np.ascontiguousarray(x[b].T.astype(bf)),
            "condT": np.ascontiguousarray(cond[b].T.astype(bf)),
            "wq": np.ascontiguousarray(wq_s[:, cols]),
            "wk": np.ascontiguousarray(wk_c[:, cols]),
            "wv": np.ascontiguousarray(wv_c[:, cols]),
            "wo": np.ascontiguousarray(wo_c[cols, :]),
        })
    return in_maps


def kernel(x, cond, Wq, Wk, Wv, Wo, bo, _trace=False, _trace_kwargs=None):
    x = np.asarray(x, dtype=np.float32)
    cond = np.asarray(cond, dtype=np.float32)
    nc = _get_nc()
    in_maps = make_in_maps(x, cond,
                           np.asarray(Wq, np.float32), np.asarray(Wk, np.float32),
                           np.asarray(Wv, np.float32), np.asarray(Wo, np.float32))
    kw = {}
    if _trace:
        kw = {"trace": True, "trace_kwargs": _trace_kwargs or {}}
    res = run_bass_kernel_spmd(nc, in_maps, list(range(NCORES)), **kw)
    bo_f = np.asarray(bo, np.float32).reshape(1, D)
    outp = np.empty((B, NQ, D), dtype=np.float32)
    for b in range(B):
        outp[b] = res.results[2 * b]["out"] + res.results[2 * b + 1]["out"] + bo_f
    if _trace:
        return outp, res
    return outp


if __name__ == "__main__":
    # quick numeric self-check against numpy (no jax needed)
    rng = np.random.default_rng(0)
    s = 0.02
    x = rng.standard_normal((B, NQ, D), dtype=np.float32)
    cond = rng.standard_normal((B, NK, D), dtype=np.float32)
    Wq = (rng.standard_normal((D, D), dtype=np.float32) * s)
    Wk = (rng.standard_normal((D, D), dtype=np.float32) * s)
    Wv = (rng.standard_normal((D, D), dtype=np.float32) * s)
    Wo = (rng.standard_normal((D, D), dtype=np.float32) * s)
    bo = (rng.standard_normal((D,), dtype=np.float32) * s)

    def ref_np(x, cond):
        q = (x @ Wq).reshape(B, NQ, H, DH).transpose(0, 2, 1, 3)
        k = (cond @ Wk).reshape(B, NK, H, DH).transpose(0, 2, 1, 3)
        v = (cond @ Wv).reshape(B, NK, H, DH).transpose(0, 2, 1, 3)
        sim = np.einsum('bhid,bhjd->bhij', q, k) * SCALE
        sim = sim - sim.max(axis=-1, keepdims=True)
        a = np.exp(sim)
        a = a / a.sum(axis=-1, keepdims=True)
        o = np.einsum('bhij,bhjd->bhid', a, v)
        o = o.transpose(0, 2, 1, 3).reshape(B, NQ, D)
        return o @ Wo + bo

    import time
    t0 = time.time()
    got = kernel(x=x, cond=cond, Wq=Wq, Wk=Wk, Wv=Wv, Wo=Wo, bo=bo)
    print(f"kernel run {time.time()-t0:.1f}s")
    exp = ref_np(x.astype(np.float64), cond.astype(np.float64))
    err = np.abs(got - exp)
    rel = np.linalg.norm(got - exp) / np.linalg.norm(exp)
    print(f"rel_l2={rel:.3e} absmax_rel={err.max()/np.abs(exp).max():.3e}")


# revision 27
# speedup vs baseline: 1.3442x; 1.0409x over previous
"""CrossAttention TRN2 kernel.

Problem (hardcoded shapes):
  x    [4, 2048, 1024], cond [4, 2048, 1024]
  Wq/Wk/Wv [1024, 1024], Wo [1024, 1024], bo [1024]
  out = softmax((x@Wq) per 8 heads of 128 @ (cond@Wk)^T * 0.125) @ (cond@Wv) @ Wo + bo

Sharding: 8 cores = (batch b in 0..3) x (head-half hh in 0..1).
Each core computes heads hh*4..hh*4+3 over ALL 2048 query rows of one batch.
K/V/Q projections only cover the core's 4 heads (inner cols hh*512..hh*512+512),
so no projection work is replicated. The out-projection contracts only the
core's 512 inner dims, producing a PARTIAL [2048, 1024] output; the host sums
the two partials of each batch and adds bo. No collectives.

All matmul operands are bf16 (fp32 PSUM accumulation): full PE rate, half
LDWEIGHTS/SBUF/DMA cost vs fp32r. Scores stay transposed [j, i] so the
softmax denominator is a partition reduction (Pool engine) of the bf16
exp tiles accumulated on DVE (2x bf16 mode).

Per-head software pipeline: emit K-proj(h+1) after attention(h) so its
matmuls fill the PE stalls left by the exp (Act) dependency; out-proj of
the first i-half starts while the last head's second i-half attention runs.
"""
import numpy as np
import ml_dtypes

import concourse.bass as bass
import concourse.bacc as bacc
import concourse.tile as tile
from concourse import bass_isa, mybir
from concourse.bass_utils import run_bass_kernel_spmd

F32 = mybir.dt.float32
BF16 = mybir.dt.bfloat16
EXP = mybir.ActivationFunctionType.Exp

B, NQ, NK, D = 4, 2048, 2048, 1024   # D = query_dim = cond_dim = inner_dim = out_dim
H, DH = 8, 128                        # heads, per-head dim
HPC = 4                               # heads per core
GW = HPC * DH                         # 512 inner cols per core
SCALE = 64 ** -0.5                    # reference uses dim_head=64 for the scale
NCORES = 8
KT = D // 128                         # contraction tiles (8)
JT = NK // 128                        # key tiles (16)
IC = 1024                             # i-chunk per attention block
NIC = NQ // IC                        # 2


def build_nc():
    nc = bacc.Bacc()
    xT = nc.declare_dram_parameter("xT", [D, NQ], BF16, isOutput=False)
    condT = nc.declare_dram_parameter("condT", [D, NK], BF16, isOutput=False)
    wq = nc.declare_dram_parameter("wq", [D, GW], BF16, isOutput=False)
    wk = nc.declare_dram_parameter("wk", [D, GW], BF16, isOutput=False)
    wv = nc.declare_dram_parameter("wv", [D, GW], BF16, isOutput=False)
    wo = nc.declare_dram_parameter("wo", [GW, D], BF16, isOutput=False)
    out = nc.declare_dram_parameter("out", [NQ, D], F32, isOutput=True)

    with tile.TileContext(nc) as tc:
        with (
            nc.allow_low_precision(reason="bf16 matmul/softmax path is intended"),
            tc.tile_pool(name="wts", bufs=1) as wts,
            tc.tile_pool(name="big", bufs=1) as big,
            tc.tile_pool(name="xstream", bufs=3) as xstream,  # [128,KT,512] tiles
            tc.tile_pool(name="expp", bufs=8) as expp,
            tc.tile_pool(name="denp", bufs=2) as denp,
            tc.tile_pool(name="ostage", bufs=2) as ostage,
            tc.tile_pool(name="ps", bufs=1, space="PSUM") as ps,
        ):
            # resident weights + condT, DMA'd in consumption order:
            # k_proj(0) streams (wk[k], ct[k]) pairs, then v needs wv,
            # then q needs wq; wo only at the very end.
            wq_s = wts.tile([128, KT, GW], BF16, tag="wq_s")
            wk_s = wts.tile([128, KT, GW], BF16, tag="wk_s")
            wv_s = wts.tile([128, KT, GW], BF16, tag="wv_s")
            wo_s = wts.tile([128, HPC, D], BF16, tag="wo_s")
            ct = big.tile([128, KT, NK], BF16, tag="ct")
            for k in range(KT):
                rows = slice(k * 128, (k + 1) * 128)
                nc.sync.dma_start(out=wk_s[:, k, :], in_=wk[rows, :])
                nc.sync.dma_start(out=ct[:, k, :], in_=condT[rows, :])
            for k in range(KT):
                rows = slice(k * 128, (k + 1) * 128)
                nc.sync.dma_start(out=wv_s[:, k, :], in_=wv[rows, :])
            for k in range(KT):
                rows = slice(k * 128, (k + 1) * 128)
                nc.sync.dma_start(out=wq_s[:, k, :], in_=wq[rows, :])
            for h in range(HPC):
                nc.sync.dma_start(out=wo_s[:, h, :],
                                  in_=wo[h * 128:(h + 1) * 128, :])

            # all-ones stationary: den-matmul reduces over partitions AND
            # broadcasts the result to all 128 output partitions in one op
            ones_t = wts.tile([128, 128], BF16, tag="ones_t")
            nc.vector.memset(ones_t, 1.0)

            # per-head projections, attention output (all bf16, dh on partitions)
            kT_all = big.tile([128, HPC, NK], BF16, tag="kT_all")
            qT_all = big.tile([128, HPC, NQ], BF16, tag="qT_all")
            v_all = big.tile([128, JT, GW], BF16, tag="v_all")
            attT = big.tile([128, HPC, NQ], BF16, tag="attT")

            def k_proj(h):
                # kT_all[:, h, :] = (Wk_h)^T @ condT ; [dh=128, j=2048]
                # [128,512] accs through the depth-2 fill ring: next acc's
                # matmuls overlap the previous acc's copy-out.
                for jh in range(NK // 512):
                    acc = ps.tile([128, 512], F32, tag="fill", bufs=2,
                                  name=f"kacc_{h}_{jh}")
                    for k in range(KT):
                        nc.tensor.matmul(
                            acc,
                            wk_s[:, k, h * DH:(h + 1) * DH],
                            ct[:, k, jh * 512:(jh + 1) * 512],
                            start=(k == 0), stop=(k == KT - 1))
                    nc.vector.tensor_copy(
                        kT_all[:, h, jh * 512:(jh + 1) * 512], acc)

            def q_proj_heads(hp):
                # qT_all for heads {2hp, 2hp+1}; streams xT (re-read per hp)
                for ih in range(NQ // 512):
                    xk = xstream.tile([128, KT, 512], BF16, tag="xk",
                                      name=f"xk_{hp}_{ih}")
                    for k in range(KT):
                        nc.sync.dma_start(
                            out=xk[:, k, :],
                            in_=xT[k * 128:(k + 1) * 128, ih * 512:(ih + 1) * 512])
                    for hh in range(2):
                        h = hp * 2 + hh
                        acc = ps.tile([128, 512], F32, tag="fill", bufs=2,
                                      name=f"qacc_{hp}_{ih}_{hh}")
                        for k in range(KT):
                            nc.tensor.matmul(
                                acc,
                                wq_s[:, k, h * DH:(h + 1) * DH],
                                xk[:, k, :],
                                start=(k == 0), stop=(k == KT - 1))
                        nc.vector.tensor_copy(
                            qT_all[:, h, ih * 512:(ih + 1) * 512], acc)

            def v_proj_all():
                # v_all[:, jt, :] = condT_jt^T @ Wv (all 4 heads); [j=128, 512]
                for jp in range(JT // 2):
                    acc = ps.tile([128, 1024], F32, tag="sc", bufs=2,
                                  name=f"vacc_{jp}")
                    for half in range(2):
                        jt = jp * 2 + half
                        for k in range(KT):
                            nc.tensor.matmul(
                                acc[:, half * 512:(half + 1) * 512],
                                ct[:, k, jt * 128:(jt + 1) * 128],
                                wv_s[:, k, :],
                                start=(k == 0), stop=(k == KT - 1))
                    for half in range(2):
                        nc.vector.tensor_copy(
                            v_all[:, jp * 2 + half, :],
                            acc[:, half * 512:(half + 1) * 512])

            def attention(h):
                for ic in range(NIC):
                    i0 = ic * IC
                    avs = ps.tile([128, IC], F32, tag="av", bufs=1,
                                  name=f"avs_{h}_{ic}")
                    den_s = denp.tile([128, IC], BF16, tag="den_s",
                                      name=f"den_s_{h}_{ic}")
                    for jt in range(JT):
                        sc = ps.tile([128, IC], F32, tag="sc", bufs=2)
                        for ih in range(IC // 512):
                            nc.tensor.matmul(
                                sc[:, ih * 512:(ih + 1) * 512],
                                kT_all[:, h, jt * 128:(jt + 1) * 128],
                                qT_all[:, h, i0 + ih * 512:i0 + (ih + 1) * 512],
                                start=True, stop=True)
                        esc = expp.tile([128, IC], BF16, tag="esc")
                        nc.scalar.activation(esc, sc, EXP)
                        for ih in range(IC // 512):
                            nc.tensor.matmul(
                                avs[:, ih * 512:(ih + 1) * 512],
                                v_all[:, jt, h * DH:(h + 1) * DH],
                                esc[:, ih * 512:(ih + 1) * 512],
                                start=(jt == 0), stop=(jt == JT - 1))
                        if jt == 0:
                            nc.vector.tensor_copy(den_s, esc)
                        else:
                            nc.vector.tensor_add(den_s, den_s, esc)
                    dsum = ps.tile([128, IC], F32, tag="sc", bufs=2,
                                   name=f"dsum_{h}_{ic}")
                    for ih in range(IC // 512):
                        nc.tensor.matmul(
                            dsum[:, ih * 512:(ih + 1) * 512],
                            ones_t,
                            den_s[:, ih * 512:(ih + 1) * 512],
                            start=True, stop=True)
                    den_bc = denp.tile([128, IC], F32, tag="den_bc",
                                       name=f"den_bc_{h}_{ic}")
                    nc.vector.reciprocal_approx_fast(den_bc, dsum)
                    nc.vector.tensor_mul(
                        attT[:, h, i0:i0 + IC], avs, den_bc)

            def out_proj(it):
                # out[it*128:(it+1)*128, :] = attT_it^T @ Wo  (partial; host adds bo)
                # even it: two [128,512] tiles through the fill ring (runs as
                # attention filler); odd it: one [128,1024] tile on the av ring
                # (free once the final avs has drained).
                fo = ostage.tile([128, D], F32, tag="fo")
                for nh in range(D // 512):
                    ot = ps.tile([128, 512], F32, tag="fill", bufs=2,
                                 name=f"ot_{it}_{nh}")
                    for h in range(HPC):
                        nc.tensor.matmul(
                            ot,
                            attT[:, h, it * 128:(it + 1) * 128],
                            wo_s[:, h, nh * 512:(nh + 1) * 512],
                            start=(h == 0), stop=(h == HPC - 1))
                    nc.vector.tensor_copy(
                        fo[:, nh * 512:(nh + 1) * 512], ot)
                nc.sync.dma_start(
                    out=out[it * 128:(it + 1) * 128, :], in_=fo)

            # emission order = scheduler priority: attention(h) emitted before
            # later projections so those matmuls fill attention's Act stalls.
            k_proj(0)
            q_proj_heads(0)
            v_proj_all()
            attention(0)
            k_proj(1)
            attention(1)
            k_proj(2)
            q_proj_heads(1)    # filler during A1; ready before A2 needs it
            attention(2)
            k_proj(3)
            attention(3)
            for it in range(NQ // 128):
                out_proj(it)
    nc.finalize()
    return nc


_NC_CACHE = None


def _get_nc():
    global _NC_CACHE
    if _NC_CACHE is None:
        _NC_CACHE = build_nc()
    return _NC_CACHE


def make_in_maps(x, cond, Wq, Wk, Wv, Wo):
    bf = ml_dtypes.bfloat16
    wq_s = (Wq.astype(np.float64) * SCALE).astype(bf)
    wk_c = Wk.astype(bf)
    wv_c = Wv.astype(bf)
    wo_c = Wo.astype(bf)
    in_maps = []
    for c in range(NCORES):
        b, hh = c // 2, c % 2
        cols = slice(hh * GW, (hh + 1) * GW)
        in_maps.append({
            "xT": np.ascontiguousarray(x[b].T.astype(bf)),
            "condT": np.ascontiguousarray(cond[b].T.astype(bf)),
            "wq": np.ascontiguousarray(wq_s[:, cols]),
            "wk": np.ascontiguousarray(wk_c[:, cols]),
            "wv": np.ascontiguousarray(wv_c[:, cols]),
            "wo": np.ascontiguousarray(wo_c[cols, :]),
        })
    return in_maps


def kernel(x, cond, Wq, Wk, Wv, Wo, bo, _trace=False, _trace_kwargs=None):
    x = np.asarray(x, dtype=np.float32)
    cond = np.asarray(cond, dtype=np.float32)
    nc = _get_nc()
    in_maps = make_in_maps(x, cond,
                           np.asarray(Wq, np.float32), np.asarray(Wk, np.float32),
                           np.asarray(Wv, np.float32), np.asarray(Wo, np.float32))
    kw = {}
    if _trace:
        kw = {"trace": True, "trace_kwargs": _trace_kwargs or {}}
    res = run_bass_kernel_spmd(nc, in_maps, list(range(NCORES)), **kw)
    bo_f = np.asarray(bo, np.float32).reshape(1, D)
    outp = np.empty((B, NQ, D), dtype=np.float32)
    for b in range(B):
        outp[b] = res.results[2 * b]["out"] + res.results[2 * b + 1]["out"] + bo_f
    if _trace:
        return outp, res
    return outp


if __name__ == "__main__":
    # quick numeric self-check against numpy (no jax needed)
    rng = np.random.default_rng(0)
    s = 0.02
    x = rng.standard_normal((B, NQ, D), dtype=np.float32)
    cond = rng.standard_normal((B, NK, D), dtype=np.float32)
    Wq = (rng.standard_normal((D, D), dtype=np.float32) * s)
    Wk = (rng.standard_normal((D, D), dtype=np.float32) * s)
    Wv = (rng.standard_normal((D, D), dtype=np.float32) * s)
    Wo = (rng.standard_normal((D, D), dtype=np.float32) * s)
    bo = (rng.standard_normal((D,), dtype=np.float32) * s)

    def ref_np(x, cond):
        q = (x @ Wq).reshape(B, NQ, H, DH).transpose(0, 2, 1, 3)
        k = (cond @ Wk).reshape(B, NK, H, DH).transpose(0, 2, 1, 3)
        v = (cond @ Wv).reshape(B, NK, H, DH).transpose(0, 2, 1, 3)
        sim = np.einsum('bhid,bhjd->bhij', q, k) * SCALE
        sim = sim - sim.max(axis=-1, keepdims=True)
        a = np.exp(sim)
        a = a / a.sum(axis=-1, keepdims=True)
        o = np.einsum('bhij,bhjd->bhid', a, v)
        o = o.transpose(0, 2, 1, 3).reshape(B, NQ, D)
        return o @ Wo + bo

    import time
    t0 = time.time()
    got = kernel(x=x, cond=cond, Wq=Wq, Wk=Wk, Wv=Wv, Wo=Wo, bo=bo)
    print(f"kernel run {time.time()-t0:.1f}s")
    exp = ref_np(x.astype(np.float64), cond.astype(np.float64))
    err = np.abs(got - exp)
    rel = np.linalg.norm(got - exp) / np.linalg.norm(exp)
    print(f"rel_l2={rel:.3e} absmax_rel={err.max()/np.abs(exp).max():.3e}")
